# revision 7
# baseline (speedup 1.0000x reference)
"""DeepSeek sparse attention on 8 Trainium2 NeuronCores.

Head-sharded (2 heads/core). Per core:
  - indexer scores X = iq @ ik^T in PE-fp32 (selection-grade precision)
  - exact-ish top-32 per query via 16-sublattice max8 + 4x(max8+match_replace)
  - mask M = (X >= t32) built per-partition, DMA-transposed to [s,q]
  - attention in [s,q] layout: E^T = exp(S^T), W^T = E^T * M^T (GPSIMD),
    AV via ones-augmented V (free softmax denominators), normalize in [q,dh],
    out_proj partial per core; host sums the 8 partials.
"""
import sys

sys.path.insert(0, '/opt/trn_rl_repo')
sys.path.insert(0, '/opt/pypackages')

import numpy as np
import ml_dtypes

BF16 = ml_dtypes.bfloat16

B, T, D = 1, 2048, 1024
H, DH, DI, KSEL = 16, 64, 32, 32
NCORES = 8
HPC = H // NCORES          # heads per core
NT = T // 128              # 16 query/key tiles
NK = D // 128              # 8 contraction chunks

_COMPILED = {}


def _install_drain_patch():
    import concourse.mybir as mybir
    from concourse.tile import TileContext
    from concourse.vector_clock import ScopedClock

    if getattr(TileContext, "_dsa_patched", False):
        return

    def _patched(self, tick_clock, wait_clock):
        nc = self.nc
        drain_inst = nc.sync.drain()
        wait_clock.add_sem_waits(
            drain_inst.ins, ScopedClock({None: tick_clock.global_clock})
        )
        si = drain_inst.ins.sync_info
        waits = list(si.on_wait or []) if si is not None else []
        if len(waits) > 1:
            drain_inst.ins.sync_info = mybir.SyncInfo(
                on_wait=waits[:1], on_update=list(si.on_update or [])
            )
            for i in range(1, len(waits)):
                extra = nc.sync.drain()
                extra.ins.sync_info = mybir.SyncInfo(
                    on_wait=waits[i:i + 1], on_update=[]
                )
        nc.all_engine_barrier()
        assert self.sems is not None
        popped = nc._tile_sem_poison_stack.pop()
        assert popped is self._sem_poison
        nc.clear_and_free_semaphores(list(self.sems.allocated().values()))
        nc.all_engine_barrier()

    TileContext._drain_and_barrier = _patched
    TileContext._dsa_patched = True


def _split_excess_waits(nc, limit=1):
    """walrus in this container rejects instructions with more sync waits
    than the ISA struct encodes; hoist excess waits onto standalone
    EventSemaphore instructions on the same engine, inserted just before."""
    import concourse.mybir as mybir

    n_new = 0
    for bb in nc.main_func.blocks:
        insts = bb.instructions
        i = 0
        while i < len(insts):
            ins = insts[i]
            si = ins.sync_info
            waits = list(si.on_wait or []) if si is not None else []
            if len(waits) > limit:
                ins.sync_info = mybir.SyncInfo(
                    on_wait=waits[:limit], on_update=list(si.on_update or []))
                pos = i
                for j in range(limit, len(waits), limit):
                    n_new += 1
                    w = mybir.InstEventSemaphore(
                        name=f"WSPLIT-{n_new}", ins=[], outs=[])
                    w.engine = ins.engine
                    w.sync_info = mybir.SyncInfo(
                        on_wait=waits[j:j + limit], on_update=[])
                    nc.register_instruction(w, overwrite=True)
                    insts.insert(pos, w)
                    pos += 1
                    i += 1
            i += 1
    return n_new


def _build_module():
    import concourse.bass as bass
    import concourse.mybir as mybir
    from concourse.tile import TileContext

    _install_drain_patch()
    dt = mybir.dt
    nc = bass.Bass()

    hsT_f32 = nc.declare_dram_parameter("hsT_f32", [D, T], dt.float32, isOutput=False)
    hsT_bf16 = nc.declare_dram_parameter("hsT_bf16", [D, T], dt.bfloat16, isOutput=False)
    A_cat = nc.declare_dram_parameter("A_cat", [D, 128], dt.float32, isOutput=False)
    Wqk_h0 = nc.declare_dram_parameter("Wqk_h0", [D, 128], dt.bfloat16, isOutput=False)
    Wqk_h1 = nc.declare_dram_parameter("Wqk_h1", [D, 128], dt.bfloat16, isOutput=False)
    Wv_cat = nc.declare_dram_parameter("Wv_cat", [D, 128], dt.bfloat16, isOutput=False)
    WoT_cat = nc.declare_dram_parameter("WoT_cat", [128, D], dt.bfloat16, isOutput=False)
    out_part = nc.declare_dram_parameter("out_part", [T, D], dt.float32, isOutput=True)

    with TileContext(nc) as tc:
        # ----- core-lifetime SBUF state -----
        with tc.tile_pool(name="state", bufs=1) as st:
            hsb = st.tile([128, NK, T], dt.bfloat16, tag="hsb")       # resident hs^T bf16
            Iq = st.tile([64, T], dt.float32, tag="Iq")               # iq^T, rows 32h
            Ik = st.tile([64, T], dt.float32, tag="Ik")               # ik^T, rows 32h
            QT = st.tile([128, T], dt.bfloat16, tag="QT")             # Q^T, rows 64h
            KT = st.tile([128, T], dt.bfloat16, tag="KT")             # K^T, rows 64h
            VP = st.tile([128, NT, 2, 65], dt.bfloat16, tag="VP")     # V' per s-tile per head
            MT = st.tile([128, NT, NT, 128], dt.bfloat16, tag="MT")   # mask^T [p=s|j, i, q]
            ATcatT = st.tile([128, T], dt.bfloat16, tag="ATcatT")     # normalized attn^T, 2 heads
            wo = st.tile([128, D], dt.bfloat16, tag="wo")
            nc.sync.dma_start(out=hsb[:], in_=hsT_bf16[:].rearrange("(c p) t -> p c t", p=128))
            nc.sync.dma_start(out=wo[:], in_=WoT_cat[:])

            # ================= P0: projections =================
            with tc.tile_pool(name="p0s", bufs=2) as p0s, \
                 tc.tile_pool(name="p0w", bufs=1) as p0w, \
                 tc.tile_pool(name="p0p", bufs=1, space="PSUM") as p0p:
                a_w = p0w.tile([128, NK, 128], dt.float32, tag="a_w")
                qk0_w = p0w.tile([128, NK, 128], dt.bfloat16, tag="qk0_w")
                qk1_w = p0w.tile([128, NK, 128], dt.bfloat16, tag="qk1_w")
                v_w = p0w.tile([128, NK, 128], dt.bfloat16, tag="v_w")
                nc.sync.dma_start(out=a_w[:], in_=A_cat[:].rearrange("(c p) m -> p c m", p=128))
                nc.sync.dma_start(out=qk0_w[:], in_=Wqk_h0[:].rearrange("(c p) m -> p c m", p=128))
                nc.sync.dma_start(out=qk1_w[:], in_=Wqk_h1[:].rearrange("(c p) m -> p c m", p=128))
                nc.sync.dma_start(out=v_w[:], in_=Wv_cat[:].rearrange("(c p) m -> p c m", p=128))

                ip = p0p.tile([128, T], dt.float32, tag="ip")
                qp = p0p.tile([128, T], dt.float32, tag="qp")
                # pass A: I_cat (fp32) + QK_h0 (bf16)
                for k in range(NK):
                    hf = p0s.tile([128, T], dt.float32, tag="hf")
                    nc.sync.dma_start(out=hf[:], in_=hsT_f32[128 * k:128 * k + 128, :])
                    for n in range(4):
                        nc.tensor.matmul(ip[:, 512 * n:512 * n + 512],
                                         a_w[:, k, :], hf[:, 512 * n:512 * n + 512],
                                         start=(k == 0), stop=(k == NK - 1))
                    for n in range(4):
                        nc.tensor.matmul(qp[:, 512 * n:512 * n + 512],
                                         qk0_w[:, k, :], hsb[:, k, 512 * n:512 * n + 512],
                                         start=(k == 0), stop=(k == NK - 1))
                nc.scalar.copy(out=Iq[:], in_=ip[0:64, :])
                nc.scalar.copy(out=Ik[:], in_=ip[64:128, :])
                nc.scalar.copy(out=QT[0:64, :], in_=qp[0:64, :])
                nc.scalar.copy(out=KT[0:64, :], in_=qp[64:128, :])

                # pass B: QK_h1 + V (bf16)
                qp1 = p0p.tile([128, T], dt.float32, tag="ip")  # reuse slot
                vp_ps = p0p.tile([128, T], dt.float32, tag="qp")
                for k in range(NK):
                    for n in range(4):
                        nc.tensor.matmul(qp1[:, 512 * n:512 * n + 512],
                                         qk1_w[:, k, :], hsb[:, k, 512 * n:512 * n + 512],
                                         start=(k == 0), stop=(k == NK - 1))
                    for n in range(4):
                        nc.tensor.matmul(vp_ps[:, 512 * n:512 * n + 512],
                                         v_w[:, k, :], hsb[:, k, 512 * n:512 * n + 512],
                                         start=(k == 0), stop=(k == NK - 1))
                nc.scalar.copy(out=QT[64:128, :], in_=qp1[0:64, :])
                nc.scalar.copy(out=KT[64:128, :], in_=qp1[64:128, :])
                # V evac: vp_ps cols = s-tiles? No: vp_ps is [128 (2 heads x 64), 2048 s].
                # We need VP [s-part, tile, head, 65]. V^T layout -> transpose needed.
                # vp_ps rows 0:64 = V_h0^T [dh, s], rows 64:128 = V_h1^T.
                # DMA-transpose (bf16) after casting.
                vt_b = p0w.tile([128, T], dt.bfloat16, tag="vt_b")
                nc.scalar.copy(out=vt_b[:], in_=vp_ps[:])
                vq = p0w.tile([128, NT, 128], dt.bfloat16, tag="vq")
                nc.sync.dma_start_transpose(out=vq[:], in_=vt_b[:])
                # vq[p, b, c] = vt_b[c, 128b+p] -> partition p = s within tile b,
                # c = 0:64 V_h0 dh, 64:128 V_h1 dh
                for j in range(NT):
                    for h in range(2):
                        nc.vector.tensor_copy(VP[:, j, h, 0:64], vq[:, j, 64 * h:64 * h + 64])
                        nc.vector.memset(VP[:, j, h, 64:65], 1.0)

            # ================= per-head main loops =================
            for h in range(HPC):
                # ---- phase A: selection ----
                with tc.tile_pool(name="pa", bufs=2) as pa, \
                     tc.tile_pool(name="pap", bufs=2, space="PSUM") as pap:
                    for i in range(NT):
                        xp = pap.tile([128, T], dt.float32, tag="xp")
                        for n in range(4):
                            nc.tensor.matmul(
                                xp[:, 512 * n:512 * n + 512],
                                Iq[32 * h:32 * h + 32, 128 * i:128 * i + 128],
                                Ik[32 * h:32 * h + 32, 512 * n:512 * n + 512])
                        xs = pa.tile([128, T], dt.float32, tag="xs")
                        nc.scalar.copy(out=xs[:], in_=xp[:])
                        cand = pa.tile([128, 128], dt.float32, tag="cand")
                        v16 = xs[:].rearrange("p (s l) -> p l s", l=16)
                        for j in range(16):
                            nc.vector.max(out=cand[:, 8 * j:8 * j + 8], in_=v16[:, j, :])
                        mx = pa.tile([128, 8], dt.float32, tag="mx")
                        for r in range(4):
                            nc.vector.max(out=mx[:], in_=cand[:])
                            if r < 3:
                                nc.vector.match_replace(out=cand[:], in_to_replace=mx[:],
                                                        in_values=cand[:], imm_value=-1e30)
                        m = pa.tile([128, T], dt.bfloat16, tag="m")
                        nc.gpsimd.tensor_scalar(m[:], xs[:], mx[:, 7:8], scalar2=None,
                                                op0=mybir.AluOpType.is_ge)
                        nc.sync.dma_start_transpose(out=MT[:, :, i, :], in_=m[:])

                # ---- phase B: attention in [s, q] ----
                with tc.tile_pool(name="pb", bufs=3) as pb, \
                     tc.tile_pool(name="pbp", bufs=1, space="PSUM") as pbp, \
                     tc.tile_pool(name="pbav", bufs=1, space="PSUM") as pbav:
                    av = pbav.tile([65, T], dt.float32, tag="av")
                    for j in range(NT):
                        sp = pbp.tile([128, T], dt.float32, tag="sp")
                        for n in range(4):
                            nc.tensor.matmul(sp[:, 512 * n:512 * n + 512],
                                             KT[64 * h:64 * h + 64, 128 * j:128 * j + 128],
                                             QT[64 * h:64 * h + 64, 512 * n:512 * n + 512])
                        e = pb.tile([128, T], dt.bfloat16, tag="e")
                        nc.scalar.activation(out=e[:], in_=sp[:],
                                             func=mybir.ActivationFunctionType.Exp)
                        w = pb.tile([128, T], dt.bfloat16, tag="w")
                        nc.gpsimd.tensor_tensor(out=w[:], in0=e[:], in1=MT[:, j, :, :],
                                                op=mybir.AluOpType.mult)
                        for n in range(4):
                            nc.tensor.matmul(av[:, 512 * n:512 * n + 512],
                                             VP[:, j, h, :], w[:, 512 * n:512 * n + 512],
                                             start=(j == 0), stop=(j == NT - 1))

                    # ---- phase C: normalize + build attn^T ----
                    at = pb.tile([128, T], dt.bfloat16, tag="at")
                    nc.vector.memset(at[64:128, :], 0.0)
                    nc.scalar.copy(out=at[0:65, :], in_=av[:])
                    atq = pb.tile([128, NT, 128], dt.bfloat16, tag="atq")
                    nc.sync.dma_start_transpose(out=atq[:], in_=at[:])
                    # atq[p=q within tile, i, 0:64]=attn, [.., 64]=den
                    scr = pb.tile([128, 128], dt.bfloat16, tag="scr")
                    nc.vector.memset(scr[:], 0.0)
                    for i in range(NT):
                        rd = pb.tile([128, 1], dt.float32, tag="rd")
                        nc.vector.reciprocal(rd[:], atq[:, i, 64:65])
                        nc.vector.tensor_scalar(scr[:, 0:64], atq[:, i, 0:64], rd[:],
                                                scalar2=None, op0=mybir.AluOpType.mult)
                        tb = pb.tile([128, 128], dt.bfloat16, tag="tb")
                        nc.sync.dma_start_transpose(
                            out=tb[:].rearrange("p (b q) -> p b q", b=1), in_=scr[:])
                        nc.vector.tensor_copy(ATcatT[64 * h:64 * h + 64, 128 * i:128 * i + 128],
                                              tb[0:64, :])

            # ================= out_proj =================
            with tc.tile_pool(name="po", bufs=2) as po, \
                 tc.tile_pool(name="pop", bufs=2, space="PSUM") as pop:
                for i in range(NT):
                    op = pop.tile([128, D], dt.float32, tag="op")
                    for n in range(2):
                        nc.tensor.matmul(op[:, 512 * n:512 * n + 512],
                                         ATcatT[:, 128 * i:128 * i + 128],
                                         wo[:, 512 * n:512 * n + 512])
                    ob = po.tile([128, D], dt.float32, tag="ob")
                    nc.scalar.copy(out=ob[:], in_=op[:])
                    nc.sync.dma_start(out=out_part[128 * i:128 * i + 128, :], in_=ob[:])

    _split_excess_waits(nc, limit=1)
    return nc


def _prep_inputs(hidden_states, Wq, Wk, Wv, Wo, idx_wq, idx_wk):
    hs = np.asarray(hidden_states[0], np.float32)          # [T, D]
    hsT = np.ascontiguousarray(hs.T)                       # [D, T]
    hsT_b = hsT.astype(BF16)
    maps = []
    for c in range(NCORES):
        h0, h1 = 2 * c, 2 * c + 1
        Aq_parts, Ak_parts = [], []
        for hh in (h0, h1):
            Wq_h = Wq[64 * hh:64 * hh + 64, :].astype(np.float64)    # [64, D]
            Wk_h = Wk[64 * hh:64 * hh + 64, :].astype(np.float64)
            Aq_parts.append((Wq_h.T @ idx_wq[hh].astype(np.float64)).astype(np.float32))
            Ak_parts.append((Wk_h.T @ idx_wk[hh].astype(np.float64)).astype(np.float32))
        A_cat = np.concatenate(Aq_parts + Ak_parts, axis=1)  # [D, 128]

        def qk_chain(hh):
            Wq_h = Wq[64 * hh:64 * hh + 64, :]
            Wk_h = Wk[64 * hh:64 * hh + 64, :]
            return np.concatenate(
                [(Wq_h.T / np.sqrt(DH)).astype(BF16), Wk_h.T.astype(BF16)], axis=1)

        Wv_cat = np.concatenate(
            [Wv[64 * h0:64 * h0 + 64, :].T, Wv[64 * h1:64 * h1 + 64, :].T],
            axis=1).astype(BF16)                           # [D, 128]
        WoT_cat = np.ascontiguousarray(Wo[:, 64 * h0:64 * h0 + 128].T).astype(BF16)

        maps.append({
            "hsT_f32": hsT,
            "hsT_bf16": hsT_b,
            "A_cat": A_cat,
            "Wqk_h0": qk_chain(h0),
            "Wqk_h1": qk_chain(h1),
            "Wv_cat": Wv_cat,
            "WoT_cat": WoT_cat,
        })
    return maps


def kernel(hidden_states, Wq, Wk, Wv, Wo, idx_wq, idx_wk):
    from concourse.bass_utils import run_bass_kernel_spmd

    if "nc" not in _COMPILED:
        _COMPILED["nc"] = _build_module()
    nc = _COMPILED["nc"]

    in_maps = _prep_inputs(np.asarray(hidden_states), np.asarray(Wq),
                           np.asarray(Wk), np.asarray(Wv), np.asarray(Wo),
                           np.asarray(idx_wq), np.asarray(idx_wk))
    res = run_bass_kernel_spmd(nc, in_maps, core_ids=list(range(NCORES)))
    out = np.zeros((T, D), np.float32)
    for c in range(NCORES):
        out += np.asarray(res.results[c]["out_part"], np.float32)
    return out.reshape(B, T, D)


# revision 8
# speedup vs baseline: 2.1123x; 2.1123x over previous
"""DeepSeek sparse attention on 8 Trainium2 NeuronCores.

Head-sharded (2 heads/core). Per core:
  - indexer scores X = iq @ ik^T in PE-fp32 (selection-grade precision)
  - exact-ish top-32 per query via 16-sublattice max8 + 4x(max8+match_replace)
  - mask M = (X >= t32) built per-partition, DMA-transposed to [s,q]
  - attention in [s,q] layout: E^T = exp(S^T), W^T = E^T * M^T (GPSIMD),
    AV via ones-augmented V (free softmax denominators), normalize in [q,dh],
    out_proj partial per core; host sums the 8 partials.
"""
import sys

sys.path.insert(0, '/opt/trn_rl_repo')
sys.path.insert(0, '/opt/pypackages')

import numpy as np
import ml_dtypes

BF16 = ml_dtypes.bfloat16

B, T, D = 1, 2048, 1024
H, DH, DI, KSEL = 16, 64, 32, 32
NCORES = 8
HPC = H // NCORES          # heads per core
NT = T // 128              # 16 query/key tiles
NK = D // 128              # 8 contraction chunks

_COMPILED = {}


def _install_drain_patch():
    import concourse.mybir as mybir
    from concourse.tile import TileContext
    from concourse.vector_clock import ScopedClock

    if getattr(TileContext, "_dsa_patched", False):
        return

    def _patched(self, tick_clock, wait_clock):
        nc = self.nc
        drain_inst = nc.sync.drain()
        wait_clock.add_sem_waits(
            drain_inst.ins, ScopedClock({None: tick_clock.global_clock})
        )
        si = drain_inst.ins.sync_info
        waits = list(si.on_wait or []) if si is not None else []
        if len(waits) > 1:
            drain_inst.ins.sync_info = mybir.SyncInfo(
                on_wait=waits[:1], on_update=list(si.on_update or [])
            )
            for i in range(1, len(waits)):
                extra = nc.sync.drain()
                extra.ins.sync_info = mybir.SyncInfo(
                    on_wait=waits[i:i + 1], on_update=[]
                )
        nc.all_engine_barrier()
        assert self.sems is not None
        popped = nc._tile_sem_poison_stack.pop()
        assert popped is self._sem_poison
        nc.clear_and_free_semaphores(list(self.sems.allocated().values()))
        nc.all_engine_barrier()

    TileContext._drain_and_barrier = _patched
    TileContext._dsa_patched = True


def _split_excess_waits(nc, limit=1):
    """walrus in this container rejects instructions with more sync waits
    than the ISA struct encodes; hoist excess waits onto standalone
    EventSemaphore instructions on the same engine, inserted just before."""
    import concourse.mybir as mybir

    n_new = 0
    for bb in nc.main_func.blocks:
        insts = bb.instructions
        i = 0
        while i < len(insts):
            ins = insts[i]
            si = ins.sync_info
            waits = list(si.on_wait or []) if si is not None else []
            if len(waits) > limit:
                ins.sync_info = mybir.SyncInfo(
                    on_wait=waits[:limit], on_update=list(si.on_update or []))
                pos = i
                for j in range(limit, len(waits), limit):
                    n_new += 1
                    w = mybir.InstEventSemaphore(
                        name=f"WSPLIT-{n_new}", ins=[], outs=[])
                    w.engine = ins.engine
                    w.sync_info = mybir.SyncInfo(
                        on_wait=waits[j:j + limit], on_update=[])
                    nc.register_instruction(w, overwrite=True)
                    insts.insert(pos, w)
                    pos += 1
                    i += 1
            i += 1
    return n_new


def _build_module():
    import concourse.bass as bass
    import concourse.mybir as mybir
    from concourse.tile import TileContext

    _install_drain_patch()
    dt = mybir.dt
    nc = bass.Bass()

    hsT_f32 = nc.declare_dram_parameter("hsT_f32", [D, T], dt.float32, isOutput=False)
    hsT_bf16 = nc.declare_dram_parameter("hsT_bf16", [D, T], dt.bfloat16, isOutput=False)
    A_cat = nc.declare_dram_parameter("A_cat", [D, 128], dt.float32, isOutput=False)
    Wqk_h0 = nc.declare_dram_parameter("Wqk_h0", [D, 128], dt.bfloat16, isOutput=False)
    Wqk_h1 = nc.declare_dram_parameter("Wqk_h1", [D, 128], dt.bfloat16, isOutput=False)
    Wv_cat = nc.declare_dram_parameter("Wv_cat", [D, 128], dt.bfloat16, isOutput=False)
    WoT_cat = nc.declare_dram_parameter("WoT_cat", [128, D], dt.bfloat16, isOutput=False)
    out_part = nc.declare_dram_parameter("out_part", [T, D], dt.float32, isOutput=True)

    with TileContext(nc) as tc:
        # ----- core-lifetime SBUF state -----
        with tc.tile_pool(name="state", bufs=1) as st:
            hsb = st.tile([128, NK, T], dt.bfloat16, tag="hsb")       # resident hs^T bf16
            Iq = st.tile([64, T], dt.float32, tag="Iq")               # iq^T, rows 32h
            Ik = st.tile([64, T], dt.float32, tag="Ik")               # ik^T, rows 32h
            QT = st.tile([128, T], dt.bfloat16, tag="QT")             # Q^T, rows 64h
            KT = st.tile([128, T], dt.bfloat16, tag="KT")             # K^T, rows 64h
            VP = st.tile([128, NT, 2, 65], dt.bfloat16, tag="VP")     # V' per s-tile per head
            MT = st.tile([128, NT, NT, 128], dt.bfloat16, tag="MT")   # mask^T [p=s|j, i, q]
            ATcatT = st.tile([128, T], dt.bfloat16, tag="ATcatT")     # normalized attn^T, 2 heads
            wo = st.tile([128, D], dt.bfloat16, tag="wo")
            nc.sync.dma_start(out=hsb[:], in_=hsT_bf16[:].rearrange("(c p) t -> p c t", p=128))
            nc.sync.dma_start(out=wo[:], in_=WoT_cat[:])

            # ================= P0: projections =================
            with tc.tile_pool(name="p0s", bufs=2) as p0s, \
                 tc.tile_pool(name="p0w", bufs=1) as p0w, \
                 tc.tile_pool(name="p0p", bufs=1, space="PSUM") as p0p:
                a_w = p0w.tile([128, NK, 128], dt.float32, tag="a_w")
                qk0_w = p0w.tile([128, NK, 128], dt.bfloat16, tag="qk0_w")
                qk1_w = p0w.tile([128, NK, 128], dt.bfloat16, tag="qk1_w")
                v_w = p0w.tile([128, NK, 128], dt.bfloat16, tag="v_w")
                nc.sync.dma_start(out=a_w[:], in_=A_cat[:].rearrange("(c p) m -> p c m", p=128))
                nc.sync.dma_start(out=qk0_w[:], in_=Wqk_h0[:].rearrange("(c p) m -> p c m", p=128))
                nc.sync.dma_start(out=qk1_w[:], in_=Wqk_h1[:].rearrange("(c p) m -> p c m", p=128))
                nc.sync.dma_start(out=v_w[:], in_=Wv_cat[:].rearrange("(c p) m -> p c m", p=128))

                ip = p0p.tile([128, T], dt.float32, tag="ip")
                qp = p0p.tile([128, T], dt.float32, tag="qp")
                # pass A: I_cat (fp32) + QK_h0 (bf16)
                for k in range(NK):
                    hf = p0s.tile([128, T], dt.float32, tag="hf")
                    nc.sync.dma_start(out=hf[:], in_=hsT_f32[128 * k:128 * k + 128, :])
                    for n in range(4):
                        nc.tensor.matmul(ip[:, 512 * n:512 * n + 512],
                                         a_w[:, k, :], hf[:, 512 * n:512 * n + 512],
                                         start=(k == 0), stop=(k == NK - 1))
                    for n in range(4):
                        nc.tensor.matmul(qp[:, 512 * n:512 * n + 512],
                                         qk0_w[:, k, :], hsb[:, k, 512 * n:512 * n + 512],
                                         start=(k == 0), stop=(k == NK - 1))
                nc.scalar.copy(out=Iq[:], in_=ip[0:64, :])
                nc.scalar.copy(out=Ik[:], in_=ip[64:128, :])
                nc.scalar.copy(out=QT[0:64, :], in_=qp[0:64, :])
                nc.scalar.copy(out=KT[0:64, :], in_=qp[64:128, :])

                # pass B: QK_h1 + V (bf16)
                qp1 = p0p.tile([128, T], dt.float32, tag="ip")  # reuse slot
                vp_ps = p0p.tile([128, T], dt.float32, tag="qp")
                for k in range(NK):
                    for n in range(4):
                        nc.tensor.matmul(qp1[:, 512 * n:512 * n + 512],
                                         qk1_w[:, k, :], hsb[:, k, 512 * n:512 * n + 512],
                                         start=(k == 0), stop=(k == NK - 1))
                    for n in range(4):
                        nc.tensor.matmul(vp_ps[:, 512 * n:512 * n + 512],
                                         v_w[:, k, :], hsb[:, k, 512 * n:512 * n + 512],
                                         start=(k == 0), stop=(k == NK - 1))
                nc.scalar.copy(out=QT[64:128, :], in_=qp1[0:64, :])
                nc.scalar.copy(out=KT[64:128, :], in_=qp1[64:128, :])
                # V evac: vp_ps cols = s-tiles? No: vp_ps is [128 (2 heads x 64), 2048 s].
                # We need VP [s-part, tile, head, 65]. V^T layout -> transpose needed.
                # vp_ps rows 0:64 = V_h0^T [dh, s], rows 64:128 = V_h1^T.
                # DMA-transpose (bf16) after casting.
                vt_b = p0w.tile([128, T], dt.bfloat16, tag="vt_b")
                nc.scalar.copy(out=vt_b[:], in_=vp_ps[:])
                vq = p0w.tile([128, NT, 128], dt.bfloat16, tag="vq")
                nc.sync.dma_start_transpose(out=vq[:], in_=vt_b[:])
                # vq[p, b, c] = vt_b[c, 128b+p] -> partition p = s within tile b,
                # c = 0:64 V_h0 dh, 64:128 V_h1 dh
                for j in range(NT):
                    for h in range(2):
                        nc.vector.tensor_copy(VP[:, j, h, 0:64], vq[:, j, 64 * h:64 * h + 64])
                        nc.vector.memset(VP[:, j, h, 64:65], 1.0)

            # ================= per-head main loops =================
            for h in range(HPC):
                # ---- phase A: selection ----
                with tc.tile_pool(name="pa", bufs=2) as pa, \
                     tc.tile_pool(name="pap", bufs=2, space="PSUM") as pap:
                    for i in range(NT):
                        xp = pap.tile([128, T], dt.float32, tag="xp")
                        for n in range(4):
                            nc.tensor.matmul(
                                xp[:, 512 * n:512 * n + 512],
                                Iq[32 * h:32 * h + 32, 128 * i:128 * i + 128],
                                Ik[32 * h:32 * h + 32, 512 * n:512 * n + 512])
                        cand = pa.tile([128, 128], dt.float32, tag="cand")
                        v16 = xp[:].rearrange("p (s l) -> p l s", l=16)
                        for j in range(16):
                            nc.vector.max(out=cand[:, 8 * j:8 * j + 8], in_=v16[:, j, :])
                        mx = pa.tile([128, 8], dt.float32, tag="mx")
                        for r in range(4):
                            nc.vector.max(out=mx[:], in_=cand[:])
                            if r < 3:
                                nc.vector.match_replace(out=cand[:], in_to_replace=mx[:],
                                                        in_values=cand[:], imm_value=-1e30)
                        negt = pa.tile([128, 1], dt.float32, tag="negt")
                        nc.vector.tensor_scalar(negt[:], mx[:, 7:8], -1.0, scalar2=None,
                                                op0=mybir.AluOpType.mult)
                        ms = pa.tile([128, T], dt.bfloat16, tag="ms")
                        nc.scalar.activation(out=ms[:], in_=xp[:],
                                             func=mybir.ActivationFunctionType.Sign,
                                             bias=negt[:])
                        m = pa.tile([128, T], dt.bfloat16, tag="m")
                        nc.vector.tensor_scalar(m[:], ms[:], 0.0, scalar2=None,
                                                op0=mybir.AluOpType.is_ge)
                        nc.sync.dma_start_transpose(out=MT[:, :, i, :], in_=m[:])

                # ---- phase B: attention in [s, q] ----
                with tc.tile_pool(name="pb", bufs=3) as pb, \
                     tc.tile_pool(name="pbp", bufs=1, space="PSUM") as pbp, \
                     tc.tile_pool(name="pbav", bufs=1, space="PSUM") as pbav:
                    av = pbav.tile([65, T], dt.float32, tag="av")
                    for j in range(NT):
                        sp = pbp.tile([128, T], dt.float32, tag="sp")
                        for n in range(4):
                            nc.tensor.matmul(sp[:, 512 * n:512 * n + 512],
                                             KT[64 * h:64 * h + 64, 128 * j:128 * j + 128],
                                             QT[64 * h:64 * h + 64, 512 * n:512 * n + 512])
                        e = pb.tile([128, T], dt.bfloat16, tag="e")
                        nc.scalar.activation(out=e[:], in_=sp[:],
                                             func=mybir.ActivationFunctionType.Exp)
                        w = pb.tile([128, T], dt.bfloat16, tag="w")
                        nc.vector.tensor_tensor(out=w[:], in0=e[:], in1=MT[:, j, :, :],
                                                op=mybir.AluOpType.mult)
                        for n in range(4):
                            nc.tensor.matmul(av[:, 512 * n:512 * n + 512],
                                             VP[:, j, h, :], w[:, 512 * n:512 * n + 512],
                                             start=(j == 0), stop=(j == NT - 1))

                    # ---- phase C: normalize + build attn^T ----
                    at = pb.tile([128, T], dt.bfloat16, tag="at")
                    nc.vector.memset(at[64:128, :], 0.0)
                    nc.scalar.copy(out=at[0:65, :], in_=av[:])
                    atq = pb.tile([128, NT, 128], dt.bfloat16, tag="atq")
                    nc.sync.dma_start_transpose(out=atq[:], in_=at[:])
                    # atq[p=q within tile, i, 0:64]=attn, [.., 64]=den
                    scr = pb.tile([128, 128], dt.bfloat16, tag="scr")
                    nc.vector.memset(scr[:], 0.0)
                    for i in range(NT):
                        rd = pb.tile([128, 1], dt.float32, tag="rd")
                        nc.vector.reciprocal(rd[:], atq[:, i, 64:65])
                        nc.vector.tensor_scalar(scr[:, 0:64], atq[:, i, 0:64], rd[:],
                                                scalar2=None, op0=mybir.AluOpType.mult)
                        tb = pb.tile([128, 128], dt.bfloat16, tag="tb")
                        nc.sync.dma_start_transpose(
                            out=tb[:].rearrange("p (b q) -> p b q", b=1), in_=scr[:])
                        nc.vector.tensor_copy(ATcatT[64 * h:64 * h + 64, 128 * i:128 * i + 128],
                                              tb[0:64, :])

            # ================= out_proj =================
            with tc.tile_pool(name="po", bufs=2) as po, \
                 tc.tile_pool(name="pop", bufs=2, space="PSUM") as pop:
                for i in range(NT):
                    op = pop.tile([128, D], dt.float32, tag="op")
                    for n in range(2):
                        nc.tensor.matmul(op[:, 512 * n:512 * n + 512],
                                         ATcatT[:, 128 * i:128 * i + 128],
                                         wo[:, 512 * n:512 * n + 512])
                    ob = po.tile([128, D], dt.float32, tag="ob")
                    nc.scalar.copy(out=ob[:], in_=op[:])
                    nc.sync.dma_start(out=out_part[128 * i:128 * i + 128, :], in_=ob[:])

    _split_excess_waits(nc, limit=1)
    return nc


def _prep_inputs(hidden_states, Wq, Wk, Wv, Wo, idx_wq, idx_wk):
    hs = np.asarray(hidden_states[0], np.float32)          # [T, D]
    hsT = np.ascontiguousarray(hs.T)                       # [D, T]
    hsT_b = hsT.astype(BF16)
    maps = []
    for c in range(NCORES):
        h0, h1 = 2 * c, 2 * c + 1
        Aq_parts, Ak_parts = [], []
        for hh in (h0, h1):
            Wq_h = Wq[64 * hh:64 * hh + 64, :].astype(np.float64)    # [64, D]
            Wk_h = Wk[64 * hh:64 * hh + 64, :].astype(np.float64)
            Aq_parts.append((Wq_h.T @ idx_wq[hh].astype(np.float64)).astype(np.float32))
            Ak_parts.append((Wk_h.T @ idx_wk[hh].astype(np.float64)).astype(np.float32))
        A_cat = np.concatenate(Aq_parts + Ak_parts, axis=1)  # [D, 128]

        def qk_chain(hh):
            Wq_h = Wq[64 * hh:64 * hh + 64, :]
            Wk_h = Wk[64 * hh:64 * hh + 64, :]
            return np.concatenate(
                [(Wq_h.T / np.sqrt(DH)).astype(BF16), Wk_h.T.astype(BF16)], axis=1)

        Wv_cat = np.concatenate(
            [Wv[64 * h0:64 * h0 + 64, :].T, Wv[64 * h1:64 * h1 + 64, :].T],
            axis=1).astype(BF16)                           # [D, 128]
        WoT_cat = np.ascontiguousarray(Wo[:, 64 * h0:64 * h0 + 128].T).astype(BF16)

        maps.append({
            "hsT_f32": hsT,
            "hsT_bf16": hsT_b,
            "A_cat": A_cat,
            "Wqk_h0": qk_chain(h0),
            "Wqk_h1": qk_chain(h1),
            "Wv_cat": Wv_cat,
            "WoT_cat": WoT_cat,
        })
    return maps


def kernel(hidden_states, Wq, Wk, Wv, Wo, idx_wq, idx_wk):
    from concourse.bass_utils import run_bass_kernel_spmd

    if "nc" not in _COMPILED:
        _COMPILED["nc"] = _build_module()
    nc = _COMPILED["nc"]

    in_maps = _prep_inputs(np.asarray(hidden_states), np.asarray(Wq),
                           np.asarray(Wk), np.asarray(Wv), np.asarray(Wo),
                           np.asarray(idx_wq), np.asarray(idx_wk))
    res = run_bass_kernel_spmd(nc, in_maps, core_ids=list(range(NCORES)))
    out = np.zeros((T, D), np.float32)
    for c in range(NCORES):
        out += np.asarray(res.results[c]["out_part"], np.float32)
    return out.reshape(B, T, D)
